# revision 14
# baseline (speedup 1.0000x reference)
"""Trainium2 Bass kernel for additive (Bahdanau-style) attention.

Reference computation (per batch element b):
    kx = keys[b] @ Wx.T                      # [L, M]
    qh = query @ Wh.T + bh                   # [L1, M]
    g  = relu(kx[None,:,:] + qh[:,None,:])   # [L1, L, M]
    s  = g @ w                               # [L1, L]
    e  = softmax(s, axis=-1)
    out[b] = e @ values[b]                   # [L1, D]

Sharding: batch (B=8) across the 8 NeuronCores, one batch element per core.
query/Wx/Wh/bh/w are replicated (tiny).

Per-core algorithm (v8):
  - HW facts this version is built around (measured):
      * DVE tensor_scalar [128,1024] bf16 ~406 ns effective; ACT Relu ~1032;
        GPSIMD elementwise is an 8-lane software path (~15 us/unit -- unusable)
        and its SBUF traffic stalls DVE, so GPSIMD does only DMA/memset work.
      * PE HAM clock gate is binary 1.2/2.4 GHz; flips warm after ~3.4 us of
        SUSTAINED busy and cools after ~3.4 us idle.  Junk matmuls ramp it
        and small junk keeps it warm until the input DMAs land.
      * A single DMA moves ~95 GB/s and DMAs on one queue serialize, so the
        head splits tensors by need-time across the SP / ACT / GPSIMD queues:
        only kt + the m=0 slices of Wx/Wh + query are critical for the first
        g unit; the m>=1 weight slices and values stream in later.
  - g units: relu(kxT_tile + qhT[:, q]) as [128,1024] per-partition-bias ops,
    DVE tensor_scalar / ACT activation split ~186/70; m-OUTER loop so only
    m-tile 0 gates the start; kx matmuls for tile m+1 and qh matmuls for
    m+1 run inside block m on the (warm, fast) PE.
  - kx PSUM->SBUF bf16 casts: m=0 on DVE (latency), m>=1 as gpsimd-issued
    casting DMAs (SWDGE can convert dtypes) to keep DVE/ACT on g units.
  - scores: PE matmuls reduce over m (partitions); stationary operand is a
    window of a zero-padded copy of w so query (16j + c)'s score row lands at
    PSUM partition 32j + c; four concurrent column-tiled matmuls
    (tile_position (0,32j)) stream four g tensors at once.
  - softmax without max-subtraction (scores are O(1)); Exp writes bf16 e;
    bf16 single-pass transposes (bf16 identity); row sums via DVE reduces;
    bf16 e.T @ bf16 values matmul; 1/sum row-scale + output DMA in halves.
  - The 64 unused PSUM rows carry garbage that never reaches the output: the
    host gathers the 64 valid rows (ROW_OF_Q) from the padded per-core out.
"""

import numpy as np

import concourse.bacc as bacc
import concourse.mybir as mybir
import concourse.tile as tile
from concourse.bass_utils import run_bass_kernel_spmd
from concourse.masks import make_identity

B, L1, L, D, M = 8, 64, 1024, 512, 512
N_CORES = 8

FP32 = mybir.dt.float32
BF16 = mybir.dt.bfloat16
F32R = mybir.dt.float32r
AF = mybir.ActivationFunctionType
OP = mybir.AluOpType

NJ = 4  # column groups
NC = 16  # c values per column group (NJ * NC == L1)

N_JUNK_BIG = 8  # [128,512] cold-clock ramp matmuls (~3.6us -> HAM warm)
N_JUNK_SMALL = 14  # [128,128] warm-hold matmuls until the input DMAs land

USE_DMA_CAST = False  # PSUM is not a legal DMA source; casts stay on DVE/ACT


def _engine_of(c, j, m):
    """Static engine split for the (c, j, m) g-unit slot: DVE 189 / ACT 67."""
    if j == 3:
        return "A"
    if j == 2 and c == 15 and m < 3:
        return "A"
    return "D"


def build_kernel():
    nc = bacc.Bacc()

    keysT = nc.declare_dram_parameter("keysT", [D, L], BF16, isOutput=False)
    values = nc.declare_dram_parameter("values", [L, D], BF16, isOutput=False)
    queryT = nc.declare_dram_parameter("queryT", [D, L1], BF16, isOutput=False)
    WxT = nc.declare_dram_parameter("WxT", [D, M], BF16, isOutput=False)
    WhT = nc.declare_dram_parameter("WhT", [D, M], BF16, isOutput=False)
    bh2 = nc.declare_dram_parameter("bh2", [128, 4], FP32, isOutput=False)
    w2 = nc.declare_dram_parameter("w2", [128, 4], FP32, isOutput=False)
    out = nc.declare_dram_parameter("out", [128, D], FP32, isOutput=True)

    with tile.TileContext(nc) as tc:
        with (
            tc.tile_pool(name="const", bufs=1) as cp,
            tc.tile_pool(name="g", bufs=8) as gp,
            tc.tile_pool(name="pk", bufs=2, space="PSUM") as pp_k,
            tc.tile_pool(name="pt", bufs=2, space="PSUM") as pp_t,
            tc.tile_pool(name="po", bufs=1, space="PSUM") as pp_o,
            tc.tile_pool(name="pq", bufs=1, space="PSUM") as pp_q,
            tc.tile_pool(name="psc", bufs=1, space="PSUM") as pp_s,
        ):
            # ---- persistent SBUF tensors
            wx = cp.tile([128, 4 * M], BF16, name="wx")
            kt = cp.tile([128, 4 * L], BF16, name="kt")
            wh = cp.tile([128, 4 * M], BF16, name="wh")
            qt = cp.tile([128, 4 * L1], BF16, name="qt")
            bhs = cp.tile([128, 4], FP32, name="bhs")
            w2s = cp.tile([128, 4], FP32, name="w2s")
            vt = cp.tile([128, 8 * D], BF16, name="vt")
            kxbf = cp.tile([128, 4 * L], BF16, name="kxbf")
            qhf = cp.tile([128, 4 * L1], FP32, name="qhf")
            w2bf = cp.tile([128, 4], BF16, name="w2bf")
            wpad = cp.tile([128, 4 * 65], BF16, name="wpad")
            identb = cp.tile([128, 128], BF16, name="identb")
            e_sb = cp.tile([128, L], BF16, name="e_sb")
            eT = cp.tile([128, L], BF16, name="eT")
            ssum2 = cp.tile([128, 2], FP32, name="ssum2")
            ssum = cp.tile([128, 1], FP32, name="ssum")
            rs = cp.tile([128, 1], FP32, name="rs")
            out_sb = cp.tile([128, D], FP32, name="out_sb")
            junk_a = cp.tile([128, 128], BF16, name="junk_a")
            junk_b = cp.tile([128, 512], BF16, name="junk_b")

            # ---- input DMAs, scheduled by need-time across the three issue
            # queues (SP / ACT / GPSIMD-SWDGE).  A single DMA moves only
            # ~95 GB/s and same-queue DMAs serialize, so the first-g-unit
            # critical set (kt0, kt1, wx m-slice 0, wh m-slice 0, qt) is
            # spread over all three queues; the m>=1 weight slices follow on
            # the ACT queue (needed only ~20 us in); values is token-deferred.
            kt3 = kt[:].rearrange("p (a l2) -> p a l2", a=4)
            ktsrc = keysT.rearrange("(a p) l -> p a l", p=128)
            wx4 = wx[:].rearrange("p (a m2) -> p a m2", a=4)
            wxsrc = WxT.rearrange("(a p) m -> p a m", p=128)
            wh4 = wh[:].rearrange("p (a m2) -> p a m2", a=4)
            whsrc = WhT.rearrange("(a p) m -> p a m", p=128)

            nc.sync.dma_start(kt3[:, :, 0:512], ktsrc[:, :, 0:512])
            nc.gpsimd.dma_start(kt3[:, :, 512:1024], ktsrc[:, :, 512:1024])
            nc.scalar.dma_start(wx4[:, :, 0:128], wxsrc[:, :, 0:128])
            nc.scalar.dma_start(wh4[:, :, 0:128], whsrc[:, :, 0:128])
            nc.scalar.dma_start(
                qt[:].rearrange("p (a q2) -> p a q2", a=4),
                queryT.rearrange("(a p) q -> p a q", p=128),
            )
            nc.scalar.dma_start(bhs[:], bh2[:, :])
            nc.scalar.dma_start(w2s[:], w2[:, :])
            nc.scalar.dma_start(wx4[:, :, 128:512], wxsrc[:, :, 128:512])
            nc.scalar.dma_start(wh4[:, :, 128:512], whsrc[:, :, 128:512])

            # ---- gpsimd-side prep (engine otherwise idle in the head)
            nc.gpsimd.memset(junk_a[:], 0.0)
            nc.gpsimd.memset(junk_b[:], 0.0)
            make_identity(nc, identb[:])

            # ---- PE warm-up: ramp the HAM clock gate with big junk matmuls,
            # then hold it warm with small ones until the input DMAs land.
            # They borrow the scores PSUM buffer (overwritten by the
            # start=True of the first real score accumulation).
            pwarm = pp_s.tile([128, L], FP32, tag="ps", name="warm")
            for r in range(N_JUNK_BIG):
                nc.tensor.matmul(
                    pwarm[:, 0:512], junk_a[:], junk_b[:], start=True, stop=True
                )
            for r in range(N_JUNK_SMALL):
                nc.tensor.matmul(
                    pwarm[:, 0:128],
                    junk_a[:],
                    junk_b[:, 0:128],
                    start=True,
                    stop=True,
                )

            # ---- small prep (vector engine)
            nc.vector.tensor_copy(w2bf[:], w2s[:])
            nc.vector.memset(wpad[:], 0.0)
            for m in range(4):
                nc.vector.tensor_copy(
                    wpad[:, 65 * m + 32 : 65 * m + 33], w2bf[:, m : m + 1]
                )

            # ---- PE kx matmuls for one m-tile (PSUM); casts are emitted
            # separately (m=0 on DVE for latency, m>=1 as gpsimd casting
            # DMAs so the elementwise engines stay on g units).
            kx_psum = {}

            def kx_mm(m):
                for lc in range(2):
                    pk = pp_k.tile([128, 512], FP32, tag="pk", name=f"pk{m}{lc}")
                    for a in range(4):
                        nc.tensor.matmul(
                            pk[:],
                            wx[:, M * a + 128 * m : M * a + 128 * (m + 1)],
                            kt[:, L * a + 512 * lc : L * a + 512 * (lc + 1)],
                            start=(a == 0),
                            stop=(a == 3),
                        )
                    kx_psum[(m, lc)] = pk

            def kx_cast(m):
                for lc in range(2):
                    dst = kxbf[:, L * m + 512 * lc : L * m + 512 * (lc + 1)]
                    if m == 0 or not USE_DMA_CAST:
                        if lc == 0:
                            nc.vector.tensor_copy(dst, kx_psum[(m, lc)][:])
                        else:
                            nc.scalar.copy(dst, kx_psum[(m, lc)][:])
                    else:
                        nc.gpsimd.dma_start(dst, kx_psum[(m, lc)][:])

            # qh matmuls for one m-tile + bias-add on ACT
            pq = pp_q.tile([128, 4 * L1], FP32, tag="pq", name="pq")

            def qh_mm(m):
                for a in range(4):
                    nc.tensor.matmul(
                        pq[:, L1 * m : L1 * (m + 1)],
                        wh[:, M * a + 128 * m : M * a + 128 * (m + 1)],
                        qt[:, L1 * a : L1 * (a + 1)],
                        start=(a == 0),
                        stop=(a == 3),
                    )
                nc.scalar.activation(
                    qhf[:, L1 * m : L1 * (m + 1)],
                    pq[:, L1 * m : L1 * (m + 1)],
                    AF.Identity,
                    bias=bhs[:, m : m + 1],
                )

            qh_mm(0)
            kx_mm(0)
            kx_cast(0)

            # ---- main stage: g units (DVE+ACT) + score matmuls, m-OUTER.
            # query q = 16j + c accumulates its scores into PSUM row 32j + c.
            # kx/qh matmuls for tile m+1 run inside block m (the warm PE has
            # slack); their casts land at the top of block m+1.
            ps = pp_s.tile([128, L], FP32, tag="ps", name="ps")
            for m in range(4):
                if m > 0:
                    kx_cast(m)
                for c in range(NC):
                    if c == 6 and m < 3:
                        kx_mm(m + 1)
                    if c == 10 and m < 3:
                        qh_mm(m + 1)
                    if c == 8 and m == 0:
                        # token write creating a WAR dep that delays the 2 MB
                        # values DMA until qh m1 exists (~25 us in), keeping
                        # the head transfers uncontended
                        nc.vector.tensor_copy(vt[:, 0:1], qhf[:, 127:128])
                        nc.sync.dma_start(
                            vt[:].rearrange("p (a d2) -> p a d2", a=8),
                            values.rearrange("(a p) d -> p a d", p=128),
                        )
                    g4 = gp.tile([128, NJ * L], BF16, tag="g", name=f"g{c}_{m}")
                    kx_sl = kxbf[:, L * m : L * (m + 1)]
                    for j in range(NJ):
                        q = NC * j + c
                        gt = g4[:, L * j : L * (j + 1)]
                        if _engine_of(c, j, m) == "A":
                            nc.scalar.activation(
                                gt,
                                kx_sl,
                                AF.Relu,
                                bias=qhf[:, L1 * m + q : L1 * m + q + 1],
                            )
                        else:
                            nc.vector.tensor_scalar(
                                gt,
                                kx_sl,
                                qhf[:, L1 * m + q : L1 * m + q + 1],
                                0.0,
                                op0=OP.add,
                                op1=OP.max,
                            )
                    for lc in range(2):
                        for j in range(NJ):
                            nc.tensor.matmul(
                                ps[32 * j : 32 * (j + 1), 512 * lc : 512 * (lc + 1)],
                                wpad[:, 65 * m + 32 - c : 65 * m + 64 - c],
                                g4[:, L * j + 512 * lc : L * j + 512 * (lc + 1)],
                                start=(c == 0 and m == 0),
                                stop=(c == NC - 1 and m == 3),
                                tile_position=(0, 32 * j),
                            )

            # ---- softmax (no max subtraction; scores are O(1)) + epilogue,
            # pipelined per 128-column chunk: exp -> transpose -> copy -> mm.
            # e is bf16 (single-pass transposes, bf16 AV matmul); row sums
            # via DVE reduces.
            po = pp_o.tile([128, D], FP32, name="po")
            for a in range(8):
                if a % 2 == 0:
                    nc.scalar.activation(
                        e_sb[:, 128 * a : 128 * (a + 2)],
                        ps[:, 128 * a : 128 * (a + 2)],
                        AF.Exp,
                    )
                pt = pp_t.tile([128, 128], BF16, tag="pt", name=f"pt{a}")
                nc.tensor.transpose(pt[:], e_sb[:, 128 * a : 128 * (a + 1)], identb[:])
                nc.vector.tensor_copy(eT[:, 128 * a : 128 * (a + 1)], pt[:])
                nc.tensor.matmul(
                    po[:],
                    eT[:, 128 * a : 128 * (a + 1)],
                    vt[:, D * a : D * (a + 1)],
                    start=(a == 0),
                    stop=(a == 7),
                )
            nc.vector.reduce_sum(
                ssum2[:, 0:1], e_sb[:, 0:512], axis=mybir.AxisListType.X
            )
            nc.vector.reduce_sum(
                ssum2[:, 1:2], e_sb[:, 512:1024], axis=mybir.AxisListType.X
            )
            nc.vector.reduce_sum(ssum[:], ssum2[:], axis=mybir.AxisListType.X)
            nc.vector.reciprocal(rs[:], ssum[:])
            for h in range(2):
                nc.scalar.activation(
                    out_sb[:, 256 * h : 256 * (h + 1)],
                    po[:, 256 * h : 256 * (h + 1)],
                    AF.Copy,
                    scale=rs[:],
                )
                nc.sync.dma_start(
                    out[:, 256 * h : 256 * (h + 1)],
                    out_sb[:, 256 * h : 256 * (h + 1)],
                )

    nc.finalize()
    return nc


_NC_CACHE = {}


def get_nc():
    if "nc" not in _NC_CACHE:
        _NC_CACHE["nc"] = build_kernel()
    return _NC_CACHE["nc"]


def make_in_maps(query, keys, values, Wx, Wh, bh, w):
    import ml_dtypes

    bf16 = ml_dtypes.bfloat16
    query = np.ascontiguousarray(query, dtype=np.float32)
    keys = np.ascontiguousarray(keys, dtype=np.float32)
    values = np.ascontiguousarray(values, dtype=np.float32)
    WxT = np.ascontiguousarray(np.asarray(Wx, dtype=np.float32).T.astype(bf16))
    WhT = np.ascontiguousarray(np.asarray(Wh, dtype=np.float32).T.astype(bf16))
    queryT = np.ascontiguousarray(query.T.astype(bf16))
    bh2 = np.ascontiguousarray(np.asarray(bh, dtype=np.float32).reshape(4, 128).T)
    w2 = np.ascontiguousarray(np.asarray(w, dtype=np.float32).reshape(4, 128).T)
    in_maps = []
    for c in range(N_CORES):
        in_maps.append(
            {
                "keysT": np.ascontiguousarray(keys[c].T.astype(bf16)),
                "values": np.ascontiguousarray(values[c].astype(bf16)),
                "queryT": queryT,
                "WxT": WxT,
                "WhT": WhT,
                "bh2": bh2,
                "w2": w2,
            }
        )
    return in_maps


def run(in_maps, **kwargs):
    nc = get_nc()
    return run_bass_kernel_spmd(nc, in_maps, core_ids=list(range(N_CORES)), **kwargs)


ROW_OF_Q = np.array([32 * (q // NC) + q % NC for q in range(L1)])


def kernel(query, keys, values, Wx, Wh, bh, w):
    in_maps = make_in_maps(query, keys, values, Wx, Wh, bh, w)
    res = run(in_maps)
    return np.stack(
        [res.results[c]["out"][ROW_OF_Q, :] for c in range(N_CORES)], axis=0
    )
